# revision 1
# baseline (speedup 1.0000x reference)
"""Gaussian blur 101x101 (separable) on 4096x4096 fp32, 8 NeuronCores.

Strategy: the 2D conv kernel W = outer(gv, gh) is rank-1, so the blur is two
1D 101-tap convs. Rows are sharded 512/core; each core gets a host-prepared
padded strip (50-row halo, zero-padded edges, plus 50/78 zero columns) so the
on-device program is uniform across cores with no collectives.

Each 1D conv maps onto the PE array as banded matmuls with 128-row
contraction windows:
  pass1: tmpT[j', i] = sum_r x[r, j'] gv[r - i + 50]
         matmul(lhsT = x[rows win, cols 128a:+128], rhs = Gv_d) -> PSUM
  pass2: out[i, j] = sum_j' tmpT[j', i] gh[j' - j + 50]
         matmul(lhsT = tmpT[win a][:, 128c:+128], rhs = Gh_d) -> PSUM
with shared band tiles G_d[k, f] = g[k - f + d], d in {0, 128, 256},
f-chunks of 256 (float32r runs 1 cycle/row at moving dim >= 256).
tmpT tiles are stored at the 128-row windows pass2 needs (offset -50), so no
transposes or partition-shifts are required anywhere.
"""

import os
import time
from contextlib import ExitStack

import numpy as np

import concourse.bass as bass  # noqa: F401  (AP types come via tile/bacc)
import concourse.mybir as mybir
import concourse.tile as tile
from concourse import bacc, bass_utils

H = 4096
W = 4096
TAPS = 101
PAD = 50
N_CORES = 8
RPC = H // N_CORES          # 512 output rows per core
NW1 = 5                     # input row windows of 128 per core
XP_ROWS = 128 * NW1         # 640 = 512 + 100 halo + 28 slack (zeros)
NA = 33                     # tmpT column windows of 128
XP_COLS = 128 * NA          # 4224 = 50 + 4096 + 78 (cols incl zero pads)
FB = 256                    # band free width per matmul
DT = mybir.dt.float32

_compiled = {}


class _FastExitTC(tile.TileContext):
    """TileContext whose exit skips the per-semaphore clear storm.

    The stock exit emits dma_reset + sem_clear for every allocated semaphore
    (~250 here) plus a second all-engine barrier — ~8us of pure tail on a
    NEFF that is loaded, executed once, and unloaded. The drain + one
    barrier (which gate output-DMA completion) are kept.
    """

    def _drain_and_barrier(self, tick_clock, wait_clock):
        from concourse.vector_clock import ScopedClock

        drain_inst = self.nc.sync.drain()
        wait_clock.add_sem_waits(
            drain_inst.ins, ScopedClock({None: tick_clock.global_clock})
        )
        self.nc.all_engine_barrier()
        popped = self.nc._tile_sem_poison_stack.pop()
        assert popped is self._sem_poison


def _build_nc(mm_dtype):
    nc = bacc.Bacc(
        "TRN2",
        target_bir_lowering=False,
        debug=False,
        enable_asserts=False,
        num_devices=N_CORES,
    )
    xp = nc.dram_tensor("xp", [XP_ROWS, XP_COLS], mm_dtype, kind="ExternalInput").ap()
    bandsV = nc.dram_tensor(
        "bandsV", [128, 3 * FB], mm_dtype, kind="ExternalInput"
    ).ap()
    bandsH = nc.dram_tensor(
        "bandsH", [128, 3 * FB], mm_dtype, kind="ExternalInput"
    ).ap()
    y = nc.dram_tensor("y", [RPC, W], DT, kind="ExternalOutput").ap()

    with _FastExitTC(nc) as tc, ExitStack() as ctx:
        xw_pool = ctx.enter_context(tc.tile_pool(name="xw", bufs=1))
        band_pool = ctx.enter_context(tc.tile_pool(name="bands", bufs=1))
        tm_pool = ctx.enter_context(tc.tile_pool(name="tm", bufs=1))
        p1_pool = ctx.enter_context(tc.tile_pool(name="p1", bufs=4, space="PSUM"))
        p2_pool = ctx.enter_context(tc.tile_pool(name="p2", bufs=4, space="PSUM"))
        st_pool = ctx.enter_context(tc.tile_pool(name="st", bufs=6))

        # column-chunked window loads so pass1's first tiles aren't gated on
        # full 2.2MB window transfers; chunk order matches pass1's a-order
        ccuts = [0, 256, 640, 1280, 2304, 3328, XP_COLS]
        xw = [
            xw_pool.tile([128, XP_COLS], mm_dtype, tag=f"xw{w}", name=f"xw{w}")
            for w in range(NW1)
        ]

        # spread DMA issue over two HWDGE queues — a single queue only issues
        # one descriptor-gen op per ~600ns, which starves the PE at kernel start
        dma_engines = [nc.sync, nc.scalar]
        # PE warmup: fp32 matmuls on a DVE-memset scratch tile need no DMA,
        # so they start ~4us in and HAM reaches K=8/8 before real data lands.
        # The warmup psum shares the p2 pool's slots (released before pass 2).
        wt = band_pool.tile([128, FB], DT, tag="wt", name="wt")
        nc.vector.memset(wt[:], 0.0)
        wps = p2_pool.tile([128, FB], DT, name="wps", tag="ps2")
        for _ in range(8):
            nc.tensor.matmul(
                wps[:], lhsT=wt[:, 0:128], rhs=wt[:], start=True, stop=True
            )

        bv = band_pool.tile([128, 3 * FB], mm_dtype, tag="bv")
        nc.sync.dma_start(bv[:], bandsV[:])
        bh = band_pool.tile([128, 3 * FB], mm_dtype, tag="bh")
        nc.scalar.dma_start(bh[:], bandsH[:])
        k = 0
        for ci in range(len(ccuts) - 1):
            cs, ce = ccuts[ci], ccuts[ci + 1]
            for w in range(NW1):
                eng = dma_engines[k % 2]
                k += 1
                eng.dma_start(xw[w][:, cs:ce], xp[128 * w : 128 * (w + 1), cs:ce])

        # pass 1 and pass 2 interleaved in emission order: pass2 group t2
        # needs tm windows up to a = 4*t2 + 4, so it is emitted right after
        # that pass1 tile. The static PE schedule then backfills pass2
        # matmuls into pass1's input-DMA stalls, and output DMA overlaps
        # input DMA instead of forming a burst at the end.
        def pass2_group(t2):
            for cpt in range(RPC // 128):
                ps2 = p2_pool.tile([128, 2 * FB], DT, tag="ps2", name=f"ps2_{t2}_{cpt}")
                for hf in range(2):
                    b2 = 2 * t2 + hf
                    for ai in range(3):
                        a2 = 2 * b2 + ai
                        nc.tensor.matmul(
                            ps2[:, FB * hf : FB * (hf + 1)],
                            lhsT=tm[a2][:, 128 * cpt : 128 * (cpt + 1)],
                            rhs=bh[:, FB * ai : FB * (ai + 1)],
                            start=(ai == 0),
                            stop=(ai == 2),
                        )
                st = st_pool.tile([128, 2 * FB], DT, name=f"st_{t2}_{cpt}", tag="st")
                nc.scalar.copy(st[:], ps2[:])
                eng = dma_engines[(t2 * 4 + cpt) % 2]
                eng.dma_start(
                    y[128 * cpt : 128 * (cpt + 1), 512 * t2 : 512 * (t2 + 1)],
                    st[:],
                )

        tm = []
        for a in range(NA):
            ps1 = p1_pool.tile([128, RPC], DT, tag="ps1", name=f"ps1_{a}")
            for b in range(2):
                for di in range(3):
                    w = 2 * b + di
                    nc.tensor.matmul(
                        ps1[:, FB * b : FB * (b + 1)],
                        lhsT=xw[w][:, 128 * a : 128 * (a + 1)],
                        rhs=bv[:, FB * di : FB * (di + 1)],
                        start=(di == 0),
                        stop=(di == 2),
                    )
            tma = tm_pool.tile([128, RPC], mm_dtype, tag=f"tm{a}", name=f"tm{a}")
            nc.vector.tensor_copy(tma[:], ps1[:])
            tm.append(tma)
            if a >= 4 and a % 4 == 0:
                pass2_group(a // 4 - 1)

    nc.compile()
    return nc


def _get_nc(mm_dtype):
    key = str(mm_dtype)
    if key not in _compiled:
        _compiled[key] = _build_nc(mm_dtype)
    return _compiled[key]


def _make_band(g, d):
    # G_d[k, f] = g[k - f + d], zero outside [0, TAPS)
    idx = np.arange(128)[:, None] - np.arange(FB)[None, :] + d
    valid = (idx >= 0) & (idx < TAPS)
    return np.where(valid, g[np.clip(idx, 0, TAPS - 1)], 0.0).astype(np.float32)


def kernel(x: np.ndarray, weight: np.ndarray) -> np.ndarray:
    x = np.asarray(x, dtype=np.float32)
    Wm = np.asarray(weight, dtype=np.float32).reshape(TAPS, TAPS)
    assert x.shape == (H, W), x.shape

    # rank-1 (separable) decomposition of the 2D kernel
    u, s, vt = np.linalg.svd(Wm.astype(np.float64))
    gv = (u[:, 0] * np.sqrt(s[0]))
    gh = (vt[0] * np.sqrt(s[0]))
    if gv.sum() < 0:
        gv, gh = -gv, -gh
    gv = gv.astype(np.float32)
    gh = gh.astype(np.float32)

    bandsV = np.concatenate([_make_band(gv, d) for d in (0, 128, 256)], axis=1)
    bandsH = np.concatenate([_make_band(gh, d) for d in (0, 128, 256)], axis=1)

    # padded per-core strips: rows [r0-50, r0+590), cols [-50, 4174), zeros
    # outside the image
    in_maps = []
    for c in range(N_CORES):
        r0 = c * RPC
        xp = np.zeros((XP_ROWS, XP_COLS), np.float32)
        lo = r0 - PAD
        hi = min(r0 + RPC + PAD, H)
        src_lo = max(lo, 0)
        xp[src_lo - lo : hi - lo, PAD : PAD + W] = x[src_lo:hi]
        in_maps.append({"xp": xp, "bandsV": bandsV, "bandsH": bandsH})

    mm_dtype = (
        mybir.dt.float32
        if os.environ.get("BLUR_MM_DTYPE") == "fp32"
        else mybir.dt.float32r
    )
    nc = _get_nc(mm_dtype)

    trace = os.environ.get("BLUR_TRACE") == "1"
    res = None
    last_exc = None
    for attempt in range(3):
        try:
            res = bass_utils.run_bass_kernel_spmd(
                nc, in_maps, core_ids=list(range(N_CORES)), trace=trace
            )
            break
        except Exception as e:  # transient NRT/device blips — retry
            last_exc = e
            time.sleep(2.0)
    if res is None:
        raise last_exc
    if trace:
        print(f"HW exec time: {res.exec_time_ns} ns")
        print(f"mean exec time: {res.mean_exec_time_ns} ns")
        if res.instructions_and_trace is not None:
            print(f"trace: {res.instructions_and_trace[1]}")

    out = np.concatenate([res.results[c]["y"] for c in range(N_CORES)], axis=0)
    return out[None, None]



# revision 3
# speedup vs baseline: 1.4139x; 1.4139x over previous
"""Gaussian blur 101x101 (separable) on 4096x4096 fp32, 8 NeuronCores.

Strategy: the 2D kernel W = outer(gv, gh) is rank-1, so the blur is two 1D
101-tap convs. Rows are sharded 512/core; each core gets a host-prepared
padded fp16 strip (50-row halo, zero-padded edges) so the on-device program
is uniform across cores with no collectives.

Each 1D conv maps onto the PE array as banded matmuls with 128-row
contraction windows and 128-wide output chunks. Because TAPS=101 < 128,
each 128-output chunk needs exactly 2 contraction windows (256 cycles per
128x128 output tile — the K=128 floor). Adjacent chunks share windows, so
per window ONE "straddling" N=256 matmul writes both neighbouring chunks
at once (lower half accumulates via band G1, upper half starts via G0):
PSUM's per-element has_written bit turns first-touch into overwrite and
second-touch into accumulate, with start=True on the first matmul of the
bank marking the whole 2KB bank pending-zero.

  pass1: tmT[j', 512a + i] = sum_r x[r, j'] gv[r - i]   (5 MMs per window a)
  pass2: y[i, j] = sum_j' tmT[j', i] gh[j' - j]         (10 MMs per (g2, q))

Everything lives in fp16 (x strip, band tiles, tm intermediate, y output)
with fp32 PSUM accumulation: halves DMA traffic vs fp32 and enables fast
weight load; rel err ~1e-4 vs the 2e-2 gate.
"""

import os
import time
from contextlib import ExitStack

import numpy as np

import concourse.bass as bass  # noqa: F401  (AP types come via tile/bacc)
import concourse.mybir as mybir
import concourse.tile as tile
from concourse import bacc, bass_utils

H = 4096
W = 4096
TAPS = 101
PAD = 50
N_CORES = 8
RPC = H // N_CORES          # 512 output rows per core
NW1 = 5                     # input row windows of 128 per core
XP_ROWS = 128 * NW1         # 640 = 512 + 100 halo + 28 slack (zeros)
NA = 33                     # tmT column windows of 128
XP_COLS = 128 * NA          # 4224 = 50 + 4096 + 78 (cols incl zero pads)
DT = mybir.dt.float32

_compiled = {}


class _FastExitTC(tile.TileContext):
    """TileContext whose exit skips the per-semaphore clear storm.

    The stock exit emits dma_reset + sem_clear for every allocated semaphore
    plus a second all-engine barrier — pure tail on a NEFF that is loaded,
    executed once, and unloaded. The drain + one barrier (which gate
    output-DMA completion) are kept.
    """

    def _drain_and_barrier(self, tick_clock, wait_clock):
        from concourse.vector_clock import ScopedClock

        drain_inst = self.nc.sync.drain()
        wait_clock.add_sem_waits(
            drain_inst.ins, ScopedClock({None: tick_clock.global_clock})
        )
        self.nc.all_engine_barrier()
        popped = self.nc._tile_sem_poison_stack.pop()
        assert popped is self._sem_poison

def _build_nc(mm_dtype):
    nc = bacc.Bacc(
        "TRN2",
        target_bir_lowering=False,
        debug=False,
        enable_asserts=False,
        num_devices=N_CORES,
    )
    xp = nc.dram_tensor("xp", [XP_ROWS, XP_COLS], mm_dtype, kind="ExternalInput").ap()
    bandst = nc.dram_tensor(
        "bandst", [128, 512], mm_dtype, kind="ExternalInput"
    ).ap()
    y = nc.dram_tensor("y", [RPC, W], mm_dtype, kind="ExternalOutput").ap()

    with _FastExitTC(nc) as tc, ExitStack() as ctx:
        xw_pool = ctx.enter_context(tc.tile_pool(name="xw", bufs=1))
        band_pool = ctx.enter_context(tc.tile_pool(name="bands", bufs=1))
        tm_pool = ctx.enter_context(tc.tile_pool(name="tm", bufs=1))
        p1_pool = ctx.enter_context(tc.tile_pool(name="p1", bufs=2, space="PSUM"))
        p2_pool = ctx.enter_context(tc.tile_pool(name="p2", bufs=2, space="PSUM"))
        st_pool = ctx.enter_context(tc.tile_pool(name="st", bufs=4))

        xw = [
            xw_pool.tile([128, XP_COLS], mm_dtype, tag=f"xw{w}", name=f"xw{w}")
            for w in range(NW1)
        ]
        tm = tm_pool.tile([128, 512 * NA], mm_dtype, tag="tm", name="tm")

        # PE warmup: fp16 matmuls on a DVE-memset scratch tile need no DMA,
        # so they run while the first input windows are still in flight.
        wt = band_pool.tile([128, 512], mm_dtype, tag="wt", name="wt")
        nc.vector.memset(wt[:], 0.0)
        wps = p2_pool.tile([128, 1024], DT, name="wps", tag="ps2")
        for _ in range(8):
            nc.tensor.matmul(
                wps[:, 0:512], lhsT=wt[:, 0:128], rhs=wt[:], start=True, stop=True
            )

        # input DMA: two HWDGE queues (sync + scalar) — a single queue only
        # issues one descriptor-gen op per ~600ns, which starves the PE at
        # kernel start. Column chunks sized so the a-sweep never waits.
        bt = band_pool.tile([128, 512], mm_dtype, tag="bt", name="bt")
        nc.sync.dma_start(bt[:], bandst[:])
        dma_engines = [nc.scalar, nc.sync]
        ccuts = [0, 512, 1536, 3072, XP_COLS]
        k = 0
        for ci in range(len(ccuts) - 1):
            cs, ce = ccuts[ci], ccuts[ci + 1]
            for w in range(NW1):
                eng = dma_engines[k % 2]
                k += 1
                eng.dma_start(xw[w][:, cs:ce], xp[128 * w : 128 * (w + 1), cs:ce])

        # band tile column layout: [Gv1 | Gv0 | Gh1 | Gh0]
        GV1, GV0, GH1, GH0 = 0, 128, 256, 384

        cast_k = 0

        def cast(dst, src):
            nonlocal cast_k
            eng = [nc.vector.tensor_copy, nc.scalar.copy][cast_k % 2]
            cast_k += 1
            eng(dst, src)

        def pass1_pair(ap_idx):
            """Windows a = 2*ap_idx, 2*ap_idx+1 -> tm[:, 1024*ap_idx:+1024]."""
            a_list = [a for a in (2 * ap_idx, 2 * ap_idx + 1) if a < NA]
            wid = 512 * len(a_list)
            ps1 = p1_pool.tile([128, 1024], DT, tag="ps1", name=f"ps1_{ap_idx}")
            for half, a in enumerate(a_list):
                base = 512 * half
                nc.tensor.matmul(
                    ps1[:, base : base + 128],
                    lhsT=xw[0][:, 128 * a : 128 * (a + 1)],
                    rhs=bt[:, GV0 : GV0 + 128],
                    start=True,
                    stop=False,
                )
                for w in (1, 2, 3):
                    nc.tensor.matmul(
                        ps1[:, base + 128 * (w - 1) : base + 128 * (w + 1)],
                        lhsT=xw[w][:, 128 * a : 128 * (a + 1)],
                        rhs=bt[:, GV1 : GV1 + 256],
                        start=False,
                        stop=False,
                    )
                nc.tensor.matmul(
                    ps1[:, base + 384 : base + 512],
                    lhsT=xw[4][:, 128 * a : 128 * (a + 1)],
                    rhs=bt[:, GV1 : GV1 + 128],
                    start=False,
                    stop=True,
                )
            cast(tm[:, 1024 * ap_idx : 1024 * ap_idx + wid], ps1[:, 0:wid])

        def tmv(b, q):
            return tm[:, 512 * b + 128 * q : 512 * b + 128 * (q + 1)]

        def pass2_tile(g2, q):
            """Output y[128q:+128, 1024*g2:+1024] (two 512-col groups)."""
            ps2 = p2_pool.tile([128, 1024], DT, tag="ps2", name=f"ps2_{g2}_{q}")
            for gl in (0, 1):
                g = 2 * g2 + gl
                base = 512 * gl
                b0 = 4 * g
                nc.tensor.matmul(
                    ps2[:, base : base + 128],
                    lhsT=tmv(b0, q),
                    rhs=bt[:, GH0 : GH0 + 128],
                    start=True,
                    stop=False,
                )
                for bl in (1, 2, 3):
                    nc.tensor.matmul(
                        ps2[:, base + 128 * (bl - 1) : base + 128 * (bl + 1)],
                        lhsT=tmv(b0 + bl, q),
                        rhs=bt[:, GH1 : GH1 + 256],
                        start=False,
                        stop=False,
                    )
                nc.tensor.matmul(
                    ps2[:, base + 384 : base + 512],
                    lhsT=tmv(b0 + 4, q),
                    rhs=bt[:, GH1 : GH1 + 128],
                    start=False,
                    stop=True,
                )
            st = st_pool.tile([128, 1024], mm_dtype, name=f"st_{g2}_{q}", tag="st")
            cast(st[:], ps2[:])
            nc.sync.dma_start(
                y[128 * q : 128 * (q + 1), 1024 * g2 : 1024 * (g2 + 1)], st[:]
            )

        # pass1 pairs with pass2 rounds interleaved: round g2 needs tm
        # windows up to 8*g2+8, i.e. pair 4*g2+4. Its four q-tiles are
        # spread across subsequent pairs so PSUM bank reuse never stalls PE.
        pending = []
        for ap_idx in range(17):
            pass1_pair(ap_idx)
            if ap_idx >= 4 and ap_idx % 4 == 0:
                g2 = ap_idx // 4 - 1
                pending = [(g2, 0), (g2, 1), (g2, 2), (g2, 3)]
            if pending:
                pass2_tile(*pending.pop(0))
                if ap_idx % 4 == 0 and pending:
                    pass2_tile(*pending.pop(0))
        for t in pending:
            pass2_tile(*t)

    nc.compile()
    return nc


def _get_nc(mm_dtype):
    key = str(mm_dtype)
    if key not in _compiled:
        _compiled[key] = _build_nc(mm_dtype)
    return _compiled[key]


def _make_band(g, d):
    # G_d[r, c] = g[r - c + 128*d], zero outside [0, TAPS)
    idx = np.arange(128)[:, None] - np.arange(128)[None, :] + 128 * d
    valid = (idx >= 0) & (idx < TAPS)
    return np.where(valid, g[np.clip(idx, 0, TAPS - 1)], 0.0).astype(np.float32)


def kernel(x: np.ndarray, weight: np.ndarray) -> np.ndarray:
    x = np.asarray(x, dtype=np.float32)
    Wm = np.asarray(weight, dtype=np.float32).reshape(TAPS, TAPS)
    assert x.shape == (H, W), x.shape

    # rank-1 (separable) decomposition of the 2D kernel
    u, s, vt = np.linalg.svd(Wm.astype(np.float64))
    gv = (u[:, 0] * np.sqrt(s[0]))
    gh = (vt[0] * np.sqrt(s[0]))
    if gv.sum() < 0:
        gv, gh = -gv, -gh
    gv = gv.astype(np.float32)
    gh = gh.astype(np.float32)

    np_dt = np.float16
    bandst = np.concatenate(
        [_make_band(gv, 1), _make_band(gv, 0), _make_band(gh, 1), _make_band(gh, 0)],
        axis=1,
    ).astype(np_dt)

    # padded fp16 plane: strip for core c is rows [c*RPC, c*RPC + 640)
    xpad = np.zeros((H + 128, XP_COLS), np_dt)
    xpad[PAD : PAD + H, PAD : PAD + W] = x.astype(np_dt)
    in_maps = []
    for c in range(N_CORES):
        r0 = c * RPC
        in_maps.append(
            {"xp": xpad[r0 : r0 + XP_ROWS], "bandst": bandst}
        )

    mm_dtype = mybir.dt.float16
    nc = _get_nc(mm_dtype)

    trace = os.environ.get("BLUR_TRACE") == "1"
    res = None
    last_exc = None
    for attempt in range(3):
        try:
            res = bass_utils.run_bass_kernel_spmd(
                nc, in_maps, core_ids=list(range(N_CORES)), trace=trace
            )
            break
        except Exception as e:  # transient NRT/device blips — retry
            last_exc = e
            time.sleep(2.0)
    if res is None:
        raise last_exc
    if trace:
        print(f"HW exec time: {res.exec_time_ns} ns")
        print(f"mean exec time: {res.mean_exec_time_ns} ns")
        if res.instructions_and_trace is not None:
            print(f"trace: {res.instructions_and_trace[1]}")

    out = np.concatenate(
        [res.results[c]["y"].astype(np.float32) for c in range(N_CORES)], axis=0
    )
    return out[None, None]


# revision 4
# speedup vs baseline: 1.4591x; 1.0320x over previous
"""Gaussian blur 101x101 (separable) on 4096x4096 fp32, 8 NeuronCores.

Strategy: the 2D kernel W = outer(gv, gh) is rank-1, so the blur is two 1D
101-tap convs. Rows are sharded 512/core; each core gets a host-prepared
padded fp16 strip (50-row halo, zero-padded edges) so the on-device program
is uniform across cores with no collectives.

Each 1D conv maps onto the PE array as banded matmuls with 128-row
contraction windows and 128-wide output chunks. Because TAPS=101 < 128,
each 128-output chunk needs exactly 2 contraction windows (256 cycles per
128x128 output tile — the K=128 floor). Adjacent chunks share windows, so
per window ONE "straddling" N=256 matmul writes both neighbouring chunks
at once (lower half accumulates via band G1, upper half starts via G0):
PSUM's per-element has_written bit turns first-touch into overwrite and
second-touch into accumulate, with start=True on the first matmul of the
bank marking the whole 2KB bank pending-zero.

  pass1: tmT[j', 512a + i] = sum_r x[r, j'] gv[r - i]   (5 MMs per window a)
  pass2: y[i, j] = sum_j' tmT[j', i] gh[j' - j]         (10 MMs per (g2, q))

Everything lives in fp16 (x strip, band tiles, tm intermediate, y output)
with fp32 PSUM accumulation: halves DMA traffic vs fp32 and enables fast
weight load; rel err ~5e-4 vs the 2e-2 gate.

The input strip is relaid out chunk-major on the host (all 5 row-windows
of a column chunk contiguous) so each chunk is a single contiguous 2D DMA;
output rounds go out as one 3D-AP DMA covering all four 128-row blocks.
"""

import os
import time
from contextlib import ExitStack

import numpy as np

import concourse.bass as bass  # noqa: F401  (AP types come via tile/bacc)
import concourse.mybir as mybir
import concourse.tile as tile
from concourse import bacc, bass_utils

H = 4096
W = 4096
TAPS = 101
PAD = 50
N_CORES = 8
RPC = H // N_CORES          # 512 output rows per core
NW1 = 5                     # input row windows of 128 per core
XP_ROWS = 128 * NW1         # 640 = 512 + 100 halo + 28 slack (zeros)
NA = 33                     # tmT column windows of 128
XP_COLS = 128 * NA          # 4224 = 50 + 4096 + 78 (cols incl zero pads)
CCUTS = [0, 512, 1024, 1536, 2560, 3584, XP_COLS]
DT = mybir.dt.float32

_compiled = {}


class _FastExitTC(tile.TileContext):
    """TileContext whose exit skips the per-semaphore clear storm.

    The stock exit emits dma_reset + sem_clear for every allocated semaphore
    plus a second all-engine barrier — pure tail on a NEFF that is loaded,
    executed once, and unloaded. The drain + one barrier (which gate
    output-DMA completion) are kept.
    """

    def _drain_and_barrier(self, tick_clock, wait_clock):
        from concourse.vector_clock import ScopedClock

        drain_inst = self.nc.sync.drain()
        wait_clock.add_sem_waits(
            drain_inst.ins, ScopedClock({None: tick_clock.global_clock})
        )
        self.nc.all_engine_barrier()
        popped = self.nc._tile_sem_poison_stack.pop()
        assert popped is self._sem_poison


def _chunk_of(a):
    c0 = 128 * a
    for ci in range(len(CCUTS) - 1):
        if CCUTS[ci] <= c0 < CCUTS[ci + 1]:
            return ci
    raise AssertionError(a)


def _build_nc(mm_dtype):
    nc = bacc.Bacc(
        "TRN2",
        target_bir_lowering=False,
        debug=False,
        enable_asserts=False,
        num_devices=N_CORES,
    )
    # chunk-major relaid strip: chunk ci cols [5*cs, 5*ce) hold the 5 row
    # windows of strip cols [cs, ce) side by side
    xp = nc.dram_tensor(
        "xp", [128, NW1 * XP_COLS], mm_dtype, kind="ExternalInput"
    ).ap()
    bandst = nc.dram_tensor(
        "bandst", [128, 512], mm_dtype, kind="ExternalInput"
    ).ap()
    y = nc.dram_tensor("y", [RPC, W], mm_dtype, kind="ExternalOutput").ap()

    with _FastExitTC(nc) as tc, ExitStack() as ctx:
        xw_pool = ctx.enter_context(tc.tile_pool(name="xw", bufs=1))
        band_pool = ctx.enter_context(tc.tile_pool(name="bands", bufs=1))
        tm_pool = ctx.enter_context(tc.tile_pool(name="tm", bufs=1))
        p1_pool = ctx.enter_context(tc.tile_pool(name="p1", bufs=4, space="PSUM"))
        p2_pool = ctx.enter_context(tc.tile_pool(name="p2", bufs=2, space="PSUM"))
        st_pool = ctx.enter_context(tc.tile_pool(name="st", bufs=2))

        xw = xw_pool.tile([128, NW1 * XP_COLS], mm_dtype, tag="xw", name="xw")
        tm = tm_pool.tile([128, 512 * NA], mm_dtype, tag="tm", name="tm")

        def lhsT1(w, a):
            ci = _chunk_of(a)
            cs, ce = CCUTS[ci], CCUTS[ci + 1]
            off = 5 * cs + w * (ce - cs) + (128 * a - cs)
            return xw[:, off : off + 128]

        # PE warmup: fp16 matmuls on a DVE-memset scratch tile need no DMA,
        # so they run while the first input chunks are still in flight.
        wt = band_pool.tile([128, 512], mm_dtype, tag="wt", name="wt")
        nc.vector.memset(wt[:], 0.0)
        wps = p2_pool.tile([128, 1024], DT, name="wps", tag="ps2")
        for _ in range(8):
            nc.tensor.matmul(
                wps[:, 0:512], lhsT=wt[:, 0:128], rhs=wt[:], start=True, stop=True
            )

        bt = band_pool.tile([128, 512], mm_dtype, tag="bt", name="bt")
        nc.sync.dma_start(bt[:], bandst[:])
        for ci in range(len(CCUTS) - 1):
            cs, ce = 5 * CCUTS[ci], 5 * CCUTS[ci + 1]
            nc.sync.dma_start(xw[:, cs:ce], xp[:, cs:ce])

        # band tile column layout: [Gv1 | Gv0 | Gh1 | Gh0]
        GV1, GV0, GH1, GH0 = 0, 128, 256, 384

        cast_k = 0

        def cast(dst, src):
            nonlocal cast_k
            eng = [nc.vector.tensor_copy, nc.scalar.copy][cast_k % 2]
            cast_k += 1
            eng(dst, src)

        def pass1_a(a):
            """Window a -> tm[:, 512a:+512]."""
            ps1 = p1_pool.tile([128, 512], DT, tag="ps1", name=f"ps1_{a}")
            nc.tensor.matmul(
                ps1[:, 0:128],
                lhsT=lhsT1(0, a),
                rhs=bt[:, GV0 : GV0 + 128],
                start=True,
                stop=False,
            )
            for w in (1, 2, 3):
                nc.tensor.matmul(
                    ps1[:, 128 * (w - 1) : 128 * (w + 1)],
                    lhsT=lhsT1(w, a),
                    rhs=bt[:, GV1 : GV1 + 256],
                    start=False,
                    stop=False,
                )
            nc.tensor.matmul(
                ps1[:, 384:512],
                lhsT=lhsT1(4, a),
                rhs=bt[:, GV1 : GV1 + 128],
                start=False,
                stop=True,
            )
            cast(tm[:, 512 * a : 512 * (a + 1)], ps1[:])

        def tmv(b, q):
            return tm[:, 512 * b + 128 * q : 512 * b + 128 * (q + 1)]

        def pass2_tile(g2, q, st):
            """One 128-row block q of output cols [1024*g2, +1024)."""
            ps2 = p2_pool.tile([128, 1024], DT, tag="ps2", name=f"ps2_{g2}_{q}")
            for gl in (0, 1):
                g = 2 * g2 + gl
                base = 512 * gl
                b0 = 4 * g
                nc.tensor.matmul(
                    ps2[:, base : base + 128],
                    lhsT=tmv(b0, q),
                    rhs=bt[:, GH0 : GH0 + 128],
                    start=True,
                    stop=False,
                )
                for bl in (1, 2, 3):
                    nc.tensor.matmul(
                        ps2[:, base + 128 * (bl - 1) : base + 128 * (bl + 1)],
                        lhsT=tmv(b0 + bl, q),
                        rhs=bt[:, GH1 : GH1 + 256],
                        start=False,
                        stop=False,
                    )
                nc.tensor.matmul(
                    ps2[:, base + 384 : base + 512],
                    lhsT=tmv(b0 + 4, q),
                    rhs=bt[:, GH1 : GH1 + 128],
                    start=False,
                    stop=True,
                )
            cast(st[:, 1024 * q : 1024 * (q + 1)], ps2[:])
            if g2 == 3:
                # final round: per-q output DMA to keep the tail short
                nc.sync.dma_start(
                    y[128 * q : 128 * (q + 1), 1024 * g2 : 1024 * (g2 + 1)],
                    st[:, 1024 * q : 1024 * (q + 1)],
                )
            elif q == 3:
                # one 3D-AP DMA for the whole round: y[(q p), c] <- st[p, (q c)]
                yv = y.rearrange("(q p) c -> p q c", q=4)[
                    :, :, 1024 * g2 : 1024 * (g2 + 1)
                ]
                sv = st.rearrange("p (q c) -> p q c", q=4)
                nc.sync.dma_start(yv, sv)

        # pass1 a-units with pass2 tiles interleaved: round g2 needs tm
        # windows up to 8*g2+8; its four q-tiles drain one per a-unit so
        # PSUM bank reuse never stalls the PE.
        pending = []
        sts = {}
        for a in range(NA):
            pass1_a(a)
            if a >= 8 and a % 8 == 0:
                g2 = a // 8 - 1
                sts[g2] = st_pool.tile(
                    [128, 4096], mm_dtype, name=f"st_{g2}", tag="st"
                )
                pending += [(g2, q) for q in range(4)]
            if pending:
                g2, q = pending.pop(0)
                pass2_tile(g2, q, sts[g2])
        g2 = 3
        sts[g2] = st_pool.tile([128, 4096], mm_dtype, name=f"st_{g2}", tag="st")
        for q in range(4):
            pass2_tile(g2, q, sts[g2])

    nc.compile()
    return nc


def _get_nc(mm_dtype):
    key = str(mm_dtype)
    if key not in _compiled:
        _compiled[key] = _build_nc(mm_dtype)
    return _compiled[key]


def _make_band(g, d):
    # G_d[r, c] = g[r - c + 128*d], zero outside [0, TAPS)
    idx = np.arange(128)[:, None] - np.arange(128)[None, :] + 128 * d
    valid = (idx >= 0) & (idx < TAPS)
    return np.where(valid, g[np.clip(idx, 0, TAPS - 1)], 0.0).astype(np.float32)


def kernel(x: np.ndarray, weight: np.ndarray) -> np.ndarray:
    x = np.asarray(x, dtype=np.float32)
    Wm = np.asarray(weight, dtype=np.float32).reshape(TAPS, TAPS)
    assert x.shape == (H, W), x.shape

    # rank-1 (separable) decomposition of the 2D kernel
    u, s, vt = np.linalg.svd(Wm.astype(np.float64))
    gv = (u[:, 0] * np.sqrt(s[0]))
    gh = (vt[0] * np.sqrt(s[0]))
    if gv.sum() < 0:
        gv, gh = -gv, -gh
    gv = gv.astype(np.float32)
    gh = gh.astype(np.float32)

    np_dt = np.float16
    bandst = np.concatenate(
        [_make_band(gv, 1), _make_band(gv, 0), _make_band(gh, 1), _make_band(gh, 0)],
        axis=1,
    ).astype(np_dt)

    # padded fp16 plane; strip for core c is rows [c*RPC, c*RPC + 640),
    # then relaid chunk-major: chunk ci holds its 5 row windows side by side
    xpad = np.zeros((H + 128, XP_COLS), np_dt)
    xpad[PAD : PAD + H, PAD : PAD + W] = x.astype(np_dt)
    in_maps = []
    for c in range(N_CORES):
        r0 = c * RPC
        strip = xpad[r0 : r0 + XP_ROWS]
        xp = np.empty((128, NW1 * XP_COLS), np_dt)
        for ci in range(len(CCUTS) - 1):
            cs, ce = CCUTS[ci], CCUTS[ci + 1]
            cw = ce - cs
            for w in range(NW1):
                off = 5 * cs + w * cw
                xp[:, off : off + cw] = strip[128 * w : 128 * (w + 1), cs:ce]
        in_maps.append({"xp": xp, "bandst": bandst})

    mm_dtype = mybir.dt.float16
    nc = _get_nc(mm_dtype)

    trace = os.environ.get("BLUR_TRACE") == "1"
    res = None
    last_exc = None
    for attempt in range(3):
        try:
            res = bass_utils.run_bass_kernel_spmd(
                nc, in_maps, core_ids=list(range(N_CORES)), trace=trace
            )
            break
        except Exception as e:  # transient NRT/device blips — retry
            last_exc = e
            time.sleep(2.0)
    if res is None:
        raise last_exc
    if trace:
        print(f"HW exec time: {res.exec_time_ns} ns")
        print(f"mean exec time: {res.mean_exec_time_ns} ns")
        if res.instructions_and_trace is not None:
            print(f"trace: {res.instructions_and_trace[1]}")

    out = np.concatenate(
        [res.results[c]["y"].astype(np.float32) for c in range(N_CORES)], axis=0
    )
    return out[None, None]


# revision 8
# speedup vs baseline: 1.5139x; 1.0375x over previous
"""Gaussian blur 101x101 (separable) on 4096x4096 fp32, 8 NeuronCores.

Strategy: the 2D kernel W = outer(gv, gh) is rank-1, so the blur is two 1D
101-tap convs. Rows are sharded 512/core; each core gets a host-prepared
padded fp16 strip (50-row halo, zero-padded edges) so the on-device program
is uniform across cores with no collectives.

Each 1D conv maps onto the PE array as banded matmuls with 128-row
contraction windows and 128-wide output chunks. Because TAPS=101 < 128,
each 128-output chunk needs exactly 2 contraction windows (256 cycles per
128x128 output tile — the K=128 floor). Adjacent chunks share windows, so
per window ONE "straddling" N=256 matmul writes both neighbouring chunks
at once (lower half accumulates via band G1, upper half starts via G0):
PSUM's per-element has_written bit turns first-touch into overwrite and
second-touch into accumulate, with start=True on the first matmul of the
bank marking the whole 2KB bank pending-zero.

  pass1: tmT[j', 512a + i] = sum_r x[r, j'] gv[r - i]   (5 MMs per window a)
  pass2: y[i, j] = sum_j' tmT[j', i] gh[j' - j]         (10 MMs per (g2, q))

Everything lives in fp16 (x strip, band tiles, tm intermediate, y output)
with fp32 PSUM accumulation: halves DMA traffic vs fp32 and enables fast
weight load; rel err ~5e-4 vs the 2e-2 gate.

The input strip is relaid out chunk-major on the host (all 5 row-windows
of a column chunk contiguous) so each chunk is a single contiguous 2D DMA;
output rounds go out as one 3D-AP DMA covering all four 128-row blocks.
"""

import os
import time
from contextlib import ExitStack

import numpy as np

import concourse.bass as bass  # noqa: F401  (AP types come via tile/bacc)
import concourse.mybir as mybir
import concourse.tile as tile
from concourse import bacc, bass_utils

H = 4096
W = 4096
TAPS = 101
PAD = 50
N_CORES = 8
RPC = H // N_CORES          # 512 output rows per core
NW1 = 5                     # input row windows of 128 per core
XP_ROWS = 128 * NW1         # 640 = 512 + 100 halo + 28 slack (zeros)
NA = 33                     # tmT column windows of 128
XP_COLS = 128 * NA          # 4224 = 50 + 4096 + 78 (cols incl zero pads)
CCUTS = [0, 512, 1024, 1536, 2560, 3584, XP_COLS]
DT = mybir.dt.float32

_compiled = {}


class _FastExitTC(tile.TileContext):
    """TileContext whose exit skips the per-semaphore clear storm.

    The stock exit emits dma_reset + sem_clear for every allocated semaphore
    plus a second all-engine barrier — pure tail on a NEFF that is loaded,
    executed once, and unloaded. The drain + one barrier (which gate
    output-DMA completion) are kept.
    """

    def _drain_and_barrier(self, tick_clock, wait_clock):
        from concourse.vector_clock import ScopedClock

        drain_inst = self.nc.sync.drain()
        wait_clock.add_sem_waits(
            drain_inst.ins, ScopedClock({None: tick_clock.global_clock})
        )
        self.nc.all_engine_barrier()
        popped = self.nc._tile_sem_poison_stack.pop()
        assert popped is self._sem_poison


def _chunk_of(a):
    c0 = 128 * a
    for ci in range(len(CCUTS) - 1):
        if CCUTS[ci] <= c0 < CCUTS[ci + 1]:
            return ci
    raise AssertionError(a)


def _build_nc(mm_dtype):
    nc = bacc.Bacc(
        "TRN2",
        target_bir_lowering=False,
        debug=False,
        enable_asserts=False,
        num_devices=N_CORES,
    )
    # chunk-major relaid strip: chunk ci cols [5*cs, 5*ce) hold the 5 row
    # windows of strip cols [cs, ce) side by side
    xp = nc.dram_tensor(
        "xp", [128, NW1 * XP_COLS], mm_dtype, kind="ExternalInput"
    ).ap()
    bandst = nc.dram_tensor(
        "bandst", [128, 512], mm_dtype, kind="ExternalInput"
    ).ap()
    y = nc.dram_tensor("y", [RPC, W], mm_dtype, kind="ExternalOutput").ap()

    with _FastExitTC(nc) as tc, ExitStack() as ctx:
        xw_pool = ctx.enter_context(tc.tile_pool(name="xw", bufs=1))
        band_pool = ctx.enter_context(tc.tile_pool(name="bands", bufs=1))
        tm_pool = ctx.enter_context(tc.tile_pool(name="tm", bufs=1))
        p1_pool = ctx.enter_context(tc.tile_pool(name="p1", bufs=4, space="PSUM"))
        p2_pool = ctx.enter_context(tc.tile_pool(name="p2", bufs=3, space="PSUM"))
        st_pool = ctx.enter_context(tc.tile_pool(name="st", bufs=3))

        xw = xw_pool.tile([128, NW1 * XP_COLS], mm_dtype, tag="xw", name="xw")
        tm = tm_pool.tile([128, 512 * NA], mm_dtype, tag="tm", name="tm")

        def lhsT1(w, a):
            ci = _chunk_of(a)
            cs, ce = CCUTS[ci], CCUTS[ci + 1]
            off = 5 * cs + w * (ce - cs) + (128 * a - cs)
            return xw[:, off : off + 128]

        # PE warmup: fp16 matmuls on a DVE-memset scratch tile need no DMA,
        # so they run while the first input chunks are still in flight.
        wt = band_pool.tile([128, 512], mm_dtype, tag="wt", name="wt")
        nc.vector.memset(wt[:], 0.0)
        wps = p2_pool.tile([128, 512], DT, name="wps", tag="ps2")
        for _ in range(8):
            nc.tensor.matmul(
                wps[:], lhsT=wt[:, 0:128], rhs=wt[:], start=True, stop=True
            )

        # two HWDGE rings (sync + scalar): each queue is a gang of DMA
        # engines good for ~240GB/s; alternating all transfers across both
        # keeps the aggregate at the HBM roofline instead of one gang.
        dma_k = 0

        def dma(dst, src):
            nonlocal dma_k
            eng = [nc.sync, nc.scalar][dma_k % 2]
            dma_k += 1
            eng.dma_start(dst, src)

        bt = band_pool.tile([128, 512], mm_dtype, tag="bt", name="bt")
        dma(bt[:], bandst[:])
        for ci in range(len(CCUTS) - 1):
            cs, ce = 5 * CCUTS[ci], 5 * CCUTS[ci + 1]
            dma(xw[:, cs:ce], xp[:, cs:ce])

        # band tile column layout: [Gv1 | Gv0 | Gh1 | Gh0]
        GV1, GV0, GH1, GH0 = 0, 128, 256, 384

        cast_k = 0

        def cast(dst, src):
            nonlocal cast_k
            eng = [nc.vector.tensor_copy, nc.scalar.copy][cast_k % 2]
            cast_k += 1
            eng(dst, src)

        def pass1_a(a):
            """Window a -> tm[:, 512a:+512]."""
            ps1 = p1_pool.tile([128, 512], DT, tag="ps1", name=f"ps1_{a}")
            nc.tensor.matmul(
                ps1[:, 0:128],
                lhsT=lhsT1(0, a),
                rhs=bt[:, GV0 : GV0 + 128],
                start=True,
                stop=False,
            )
            for w in (1, 2, 3):
                nc.tensor.matmul(
                    ps1[:, 128 * (w - 1) : 128 * (w + 1)],
                    lhsT=lhsT1(w, a),
                    rhs=bt[:, GV1 : GV1 + 256],
                    start=False,
                    stop=False,
                )
            nc.tensor.matmul(
                ps1[:, 384:512],
                lhsT=lhsT1(4, a),
                rhs=bt[:, GV1 : GV1 + 128],
                start=False,
                stop=True,
            )
            cast(tm[:, 512 * a : 512 * (a + 1)], ps1[:])

        def tmv(b, q):
            return tm[:, 512 * b + 128 * q : 512 * b + 128 * (q + 1)]

        def pass2_tile(g, q, st):
            """One 128-row block q of output cols [512*g, +512)."""
            ps2 = p2_pool.tile([128, 512], DT, tag="ps2", name=f"ps2_{g}_{q}")
            b0 = 4 * g
            nc.tensor.matmul(
                ps2[:, 0:128],
                lhsT=tmv(b0, q),
                rhs=bt[:, GH0 : GH0 + 128],
                start=True,
                stop=False,
            )
            for bl in (1, 2, 3):
                nc.tensor.matmul(
                    ps2[:, 128 * (bl - 1) : 128 * (bl + 1)],
                    lhsT=tmv(b0 + bl, q),
                    rhs=bt[:, GH1 : GH1 + 256],
                    start=False,
                    stop=False,
                )
            nc.tensor.matmul(
                ps2[:, 384:512],
                lhsT=tmv(b0 + 4, q),
                rhs=bt[:, GH1 : GH1 + 128],
                start=False,
                stop=True,
            )
            cast(st[:, 512 * q : 512 * (q + 1)], ps2[:])
            if g == 7:
                # final group: per-q output DMA to keep the tail short
                dma(
                    y[128 * q : 128 * (q + 1), 512 * g : 512 * (g + 1)],
                    st[:, 512 * q : 512 * (q + 1)],
                )
            elif q == 3:
                # one 3D-AP DMA for the whole group: y[(q p), c] <- st[p, (q c)]
                yv = y.rearrange("(q p) c -> p q c", q=4)[
                    :, :, 512 * g : 512 * (g + 1)
                ]
                sv = st.rearrange("p (q c) -> p q c", q=4)
                dma(yv, sv)

        # pass1 a-units with pass2 tiles interleaved: group g needs tm
        # windows up to 4*g+4; its four q-tiles drain one per a-unit so
        # PSUM bank reuse never stalls the PE.
        pending = []
        sts = {}
        for a in range(NA):
            pass1_a(a)
            if a >= 4 and a % 4 == 0:
                g = a // 4 - 1
                sts[g] = st_pool.tile(
                    [128, 2048], mm_dtype, name=f"st_{g}", tag="st"
                )
                pending += [(g, q) for q in range(4)]
            if pending:
                g, q = pending.pop(0)
                pass2_tile(g, q, sts[g])
        for g, q in pending:
            pass2_tile(g, q, sts[g])

    nc.compile()
    return nc


def _get_nc(mm_dtype):
    key = str(mm_dtype)
    if key not in _compiled:
        _compiled[key] = _build_nc(mm_dtype)
    return _compiled[key]


def _make_band(g, d):
    # G_d[r, c] = g[r - c + 128*d], zero outside [0, TAPS)
    idx = np.arange(128)[:, None] - np.arange(128)[None, :] + 128 * d
    valid = (idx >= 0) & (idx < TAPS)
    return np.where(valid, g[np.clip(idx, 0, TAPS - 1)], 0.0).astype(np.float32)


def kernel(x: np.ndarray, weight: np.ndarray) -> np.ndarray:
    x = np.asarray(x, dtype=np.float32)
    Wm = np.asarray(weight, dtype=np.float32).reshape(TAPS, TAPS)
    assert x.shape == (H, W), x.shape

    # rank-1 (separable) decomposition of the 2D kernel
    u, s, vt = np.linalg.svd(Wm.astype(np.float64))
    gv = (u[:, 0] * np.sqrt(s[0]))
    gh = (vt[0] * np.sqrt(s[0]))
    if gv.sum() < 0:
        gv, gh = -gv, -gh
    gv = gv.astype(np.float32)
    gh = gh.astype(np.float32)

    np_dt = np.float16
    bandst = np.concatenate(
        [_make_band(gv, 1), _make_band(gv, 0), _make_band(gh, 1), _make_band(gh, 0)],
        axis=1,
    ).astype(np_dt)

    # padded fp16 plane; strip for core c is rows [c*RPC, c*RPC + 640),
    # then relaid chunk-major: chunk ci holds its 5 row windows side by side
    xpad = np.zeros((H + 128, XP_COLS), np_dt)
    xpad[PAD : PAD + H, PAD : PAD + W] = x.astype(np_dt)
    in_maps = []
    for c in range(N_CORES):
        r0 = c * RPC
        strip = xpad[r0 : r0 + XP_ROWS]
        xp = np.empty((128, NW1 * XP_COLS), np_dt)
        for ci in range(len(CCUTS) - 1):
            cs, ce = CCUTS[ci], CCUTS[ci + 1]
            cw = ce - cs
            for w in range(NW1):
                off = 5 * cs + w * cw
                xp[:, off : off + cw] = strip[128 * w : 128 * (w + 1), cs:ce]
        in_maps.append({"xp": xp, "bandst": bandst})

    mm_dtype = mybir.dt.float16
    nc = _get_nc(mm_dtype)

    trace = os.environ.get("BLUR_TRACE") == "1"
    res = None
    last_exc = None
    for attempt in range(3):
        try:
            res = bass_utils.run_bass_kernel_spmd(
                nc, in_maps, core_ids=list(range(N_CORES)), trace=trace
            )
            break
        except Exception as e:  # transient NRT/device blips — retry
            last_exc = e
            time.sleep(2.0)
    if res is None:
        raise last_exc
    if trace:
        print(f"HW exec time: {res.exec_time_ns} ns")
        print(f"mean exec time: {res.mean_exec_time_ns} ns")
        if res.instructions_and_trace is not None:
            print(f"trace: {res.instructions_and_trace[1]}")

    out = np.concatenate(
        [res.results[c]["y"].astype(np.float32) for c in range(N_CORES)], axis=0
    )
    return out[None, None]
